# revision 8
# baseline (speedup 1.0000x reference)
"""Trainium2 Bass kernel: causal multi-head attention with RoPE.

Problem: B=2, T=2048, C=1024, H=16, HD=64.
  q/k/v = x @ W{q,k,v}.T ; rope(q), rope(k)
  att = softmax(causal(q k^T / 8)) ; out = (att v) @ Wo.T

Sharding (8 cores): core i handles batch b = i//4 and head group g = i%4
(4 heads = 2 head-pairs, channel slice c in [256g, 256g+256)).
Each core computes its partial output x[b]-slice @ Wo[:, slice].T; the host
sums the 4 partials per batch (Wo row-parallel reduction done on host).

Device-side layout (per core), same math as the 193us baseline but with the
program restructured around the measured engine loads (PE 140us busy,
ACT 108us busy of which 28us was PSUM->SBUF copies):
  - ALL PSUM->SBUF copies moved from the scalar (ACT) engine to the vector
    engine; ACT now runs exp only (~80us).
  - QKV projection + RoPE emitted as per-512-column chunk units; attention
    chunks for BOTH head pairs are interleaved with those units so the ACT
    exp stream starts at ~t=17us and stays fed while the PE retires the
    remaining QKV/V work in the gaps of the ACT-paced attention stream.
  - PSUM plan: tag "big" [128,1024]f32 x2 bufs (scores st2 + QKV/V chunk
    accumulators) + tag "os" [128,1024]f32 x2 bufs (attV accumulators +
    projection accumulators) = exactly 8 banks.
  - xt is DMA'd in (column-chunk, ci) order so the first QKV chunk's
    operands arrive first.
  - v_ext ones-columns are a single strided memset; the V PSUM->SBUF
    scatter is 2 strided vector copies (even/odd heads).
Everything else (dual-half score matmuls, [V|1] ones-folding for softmax
denominators, triangular diagonal masks, no-max exp with scale=0.125,
host-side Wo row-parallel reduction) is unchanged from the baseline.
"""

import os

import numpy as np
import ml_dtypes

B, T, C, H, HD = 2, 2048, 1024, 16, 64
N_CORES = 8
GROUPS = 4  # head groups (of 4 heads) per batch
HPG = H // GROUPS  # heads per core = 4
M_CORE = HPG * HD  # 256 head channels per core
PAIRS = HPG // 2  # head pairs per core = 2
QCHUNK = 512  # q columns per attention chunk
KTILE = 128  # k rows per tile
NQC = T // QCHUNK  # 4
NT128 = T // 128  # 16

_bf16 = ml_dtypes.bfloat16

_CACHE = {}
LAST_RESULTS = None  # BassKernelResults of the most recent run (for test.py)


def _build_bass():
    """Trace the per-core Bass/Tile program (SPMD, same NEFF on all cores)."""
    from contextlib import ExitStack

    import concourse.bass as bass
    import concourse.tile as tile
    from concourse import bacc, mybir

    f32 = mybir.dt.float32
    bf16 = mybir.dt.bfloat16
    Exp = mybir.ActivationFunctionType.Exp

    nc = bacc.Bacc(
        "TRN2",
        target_bir_lowering=False,
        debug=False,
        enable_asserts=False,
        num_devices=N_CORES,
    )

    xt_d = nc.dram_tensor("xt", [C, T], bf16, kind="ExternalInput").ap()
    wq_d = nc.dram_tensor("wqt", [C, M_CORE], bf16, kind="ExternalInput").ap()
    wk_d = nc.dram_tensor("wkt", [C, M_CORE], bf16, kind="ExternalInput").ap()
    wv_d = nc.dram_tensor("wvt", [C, M_CORE], bf16, kind="ExternalInput").ap()
    wo_d = nc.dram_tensor("wot", [M_CORE, C], bf16, kind="ExternalInput").ap()
    cmap_d = nc.dram_tensor("cmap", [128, T], bf16, kind="ExternalInput").ap()
    smap_d = nc.dram_tensor("smap", [128, T], bf16, kind="ExternalInput").ap()
    out_d = nc.dram_tensor("out", [T, C], f32, kind="ExternalOutput").ap()

    NCT = C // 128  # 8 c-tiles

    with tile.TileContext(nc) as tc:
        with ExitStack() as ctx:
            consts = ctx.enter_context(tc.tile_pool(name="consts", bufs=1))
            qk_sb = ctx.enter_context(tc.tile_pool(name="qk_sb", bufs=1))
            rope_tmp = ctx.enter_context(tc.tile_pool(name="rope_tmp", bufs=2))
            att_sb = ctx.enter_context(tc.tile_pool(name="att_sb", bufs=4))
            misc_sb = ctx.enter_context(tc.tile_pool(name="misc_sb", bufs=2))
            out_sb = ctx.enter_context(tc.tile_pool(name="out_sb", bufs=4))
            ps = ctx.enter_context(tc.tile_pool(name="ps", bufs=2, space="PSUM"))

            # ---- DMA loads, in first-use order ----
            # wq first (first QKV units), then xt column-half 0 (2KB/partition
            # lines, full DMA rate), rope maps, wk, wv (v-tiles start early),
            # then xt half 1 and wo.
            wq = []
            for i in range(NCT):
                t = consts.tile([128, M_CORE], bf16, tag=f"wq{i}", name=f"wq{i}")
                nc.sync.dma_start(t[:], wq_d[i * 128 : (i + 1) * 128, :])
                wq.append(t)
            xt = [
                consts.tile([128, T], bf16, tag=f"xt{i}", name=f"xt{i}")
                for i in range(NCT)
            ]
            half = T // 2
            for i in range(NCT):
                nc.sync.dma_start(
                    xt[i][:, 0:half], xt_d[i * 128 : (i + 1) * 128, 0:half]
                )
            cmap = consts.tile([128, T], bf16, tag="cmap", name="cmap")
            nc.sync.dma_start(cmap[:], cmap_d[:])
            smap = consts.tile([128, T], bf16, tag="smap", name="smap")
            nc.sync.dma_start(smap[:], smap_d[:])
            wk = []
            for i in range(NCT):
                t = consts.tile([128, M_CORE], bf16, tag=f"wk{i}", name=f"wk{i}")
                nc.sync.dma_start(t[:], wk_d[i * 128 : (i + 1) * 128, :])
                wk.append(t)
            wv = []
            for i in range(NCT):
                t = consts.tile([128, M_CORE], bf16, tag=f"wv{i}", name=f"wv{i}")
                nc.sync.dma_start(t[:], wv_d[i * 128 : (i + 1) * 128, :])
                wv.append(t)
            for i in range(NCT):
                nc.sync.dma_start(
                    xt[i][:, half:T], xt_d[i * 128 : (i + 1) * 128, half:T]
                )
            wo = []
            for p in range(PAIRS):
                t = consts.tile([128, C], bf16, tag=f"wo{p}", name=f"wo{p}")
                nc.sync.dma_start(t[:], wo_d[p * 128 : (p + 1) * 128, :])
                wo.append(t)

            # upper-triangular (incl. diagonal) keep-mask, duplicated twice so
            # both heads' diagonal blocks mask with ONE strided DVE multiply:
            # tri2[p, g*128+y] = (p <= y)
            tri2 = consts.tile([128, 256], bf16, tag="tri2", name="tri2")
            nc.gpsimd.memset(tri2[:], 1.0)
            for g in range(2):
                nc.gpsimd.affine_select(
                    out=tri2[:, g * 128 : (g + 1) * 128],
                    in_=tri2[:, g * 128 : (g + 1) * 128],
                    compare_op=mybir.AluOpType.is_ge,
                    fill=0.0,
                    base=0,
                    pattern=[[1, 128]],
                    channel_multiplier=-1,
                )
            tri2r = tri2.rearrange("p (g y) -> p g y", g=2)

            # v_ext tiles: [128 k, 4*128]; head h occupies cols [h*128,(h+1)*128)
            # as [V_h | 1] for even h, [1 | V_h] for odd h.  Ones regions are
            # cols [64,192) u [320,448): one strided memset per tile.
            v_ext = []
            for tt in range(NT128):
                vt = qk_sb.tile([128, 4 * 128], bf16, tag=f"v{tt}", name=f"v{tt}")
                vr = vt.rearrange("p (a b) -> p a b", a=2)  # blocks at 0, 256
                nc.gpsimd.memset(vr[:, :, 64:192], 1.0)
                v_ext.append(vt)

            # ---- per-chunk QKV projection + RoPE unit emitters ----
            # raw/roped QT,KT tiles [128 m, T]; m rows: head-local channel
            # hr = r % 64, j = hr % 32, parity = hr // 32 (host permuted W rows
            # to [evens; odds] per head so rope pairing is a +-32 row shift).
            qt_raw, kt_raw, qt_r, kt_r = {}, {}, {}, {}
            for p in range(PAIRS):
                qt_raw[p] = qk_sb.tile([128, T], bf16, tag=f"qtraw{p}", name=f"qtraw{p}")
                kt_raw[p] = qk_sb.tile([128, T], bf16, tag=f"ktraw{p}", name=f"ktraw{p}")
                qt_r[p] = qk_sb.tile([128, T], bf16, tag=f"qtr{p}", name=f"qtr{p}")
                kt_r[p] = qk_sb.tile([128, T], bf16, tag=f"ktr{p}", name=f"ktr{p}")

            def qkv_unit(w, raw, p, tch):
                """One 512-col chunk: 8 accumulating matmuls -> DVE copy."""
                cs = slice(tch * QCHUNK, (tch + 1) * QCHUNK)
                psq = ps.tile([128, QCHUNK], f32, tag="big", name="ps_qk")
                for ci in range(NCT):
                    nc.tensor.matmul(
                        psq[:],
                        lhsT=w[ci][:, p * 128 : (p + 1) * 128],
                        rhs=xt[ci][:, cs],
                        start=(ci == 0),
                        stop=(ci == NCT - 1),
                    )
                nc.vector.tensor_copy(raw[:, cs], psq[:])

            RHALF = T // 2  # rope granularity: half tensor (2 QKV chunks)

            def rope_half(raw, roped, h):
                """RoPE on columns [h*RHALF, (h+1)*RHALF): 32-row block swap
                via SDMA (parallel queues, no gpsimd cost) + 2 muls + add."""
                cs = slice(h * RHALF, (h + 1) * RHALF)
                shf = rope_tmp.tile([128, RHALF], bf16, tag="shf", name="shf")
                for dst_b, src_b in ((0, 1), (1, 0), (2, 3), (3, 2)):
                    nc.sync.dma_start(
                        shf[dst_b * 32 : (dst_b + 1) * 32, :],
                        raw[src_b * 32 : (src_b + 1) * 32, cs],
                    )
                t1 = rope_tmp.tile([128, RHALF], bf16, tag="t1", name="rope_t1")
                nc.vector.tensor_mul(t1[:], raw[:, cs], cmap[:, cs])
                t2 = rope_tmp.tile([128, RHALF], bf16, tag="t2", name="rope_t2")
                nc.vector.tensor_mul(t2[:], shf[:], smap[:, cs])
                nc.vector.tensor_add(roped[:, cs], t1[:], t2[:])

            def v_unit(tt):
                """One V k-tile: 8 accumulating matmuls -> 2 strided DVE
                copies into the [V|1] layout."""
                vt = v_ext[tt]
                psv = ps.tile([128, M_CORE], f32, tag="big", name="ps_v")
                for ci in range(NCT):
                    nc.tensor.matmul(
                        psv[:],
                        lhsT=xt[ci][:, tt * 128 : (tt + 1) * 128],
                        rhs=wv[ci][:],
                        start=(ci == 0),
                        stop=(ci == NCT - 1),
                    )
                pr = psv.rearrange("p (a b) -> p a b", a=2)  # head blocks 0,128
                vr = vt.rearrange("p (a b) -> p a b", a=2)  # dst blocks 0,256
                nc.vector.tensor_copy(vr[:, :, 0:64], pr[:, :, 0:64])  # heads 0,2
                nc.vector.tensor_copy(vr[:, :, 192:256], pr[:, :, 64:128])  # 1,3

            # ---- attention (per head pair, per q chunk) ----
            att_out = []
            for p in range(PAIRS):
                ao = qk_sb.tile([128, T], bf16, tag=f"ao{p}", name=f"ao{p}")
                att_out.append(ao)

            def attn_chunk(p, j, fillers=None):
                os2 = ps.tile([128, 2 * QCHUNK], f32, tag="os", name="ps_os")
                outA = os2[:, 0:QCHUNK]   # rows 0:64 attV_A, 64:128 sums_A
                outB = os2[:, QCHUNK:]    # rows 0:64 sums_B, 64:128 attV_B
                nkt = (j + 1) * (QCHUNK // KTILE)
                for kb in range(nkt):
                    o = KTILE * kb - QCHUNK * j
                    c0 = max(o, 0)
                    qs = slice(j * QCHUNK + c0, (j + 1) * QCHUNK)
                    ks = slice(kb * KTILE, (kb + 1) * KTILE)
                    # both heads' scores in one 2-bank tile -> single exp
                    st2 = ps.tile([128, 2 * QCHUNK], f32, tag="big", name="ps_st")
                    nc.tensor.matmul(
                        st2[:, c0:QCHUNK],
                        lhsT=kt_r[p][0:64, ks],
                        rhs=qt_r[p][0:64, qs],
                        start=True,
                        stop=True,
                        tile_position=(0, 0),
                    )
                    nc.tensor.matmul(
                        st2[:, QCHUNK + c0 :],
                        lhsT=kt_r[p][64:128, ks],
                        rhs=qt_r[p][64:128, qs],
                        start=True,
                        stop=True,
                        tile_position=(64, 0),
                    )
                    att2 = att_sb.tile([128, 2 * QCHUNK], bf16, tag="att", name="att2")
                    # single exp across both banks; the [QCHUNK, QCHUNK+c0)
                    # gap holds stale-but-finite scores and is never read
                    nc.scalar.activation(att2[:, c0:], st2[:, c0:], Exp, scale=0.125)
                    if o >= 0:  # diagonal tile: triangular mask, both heads at once
                        a3 = att2.rearrange("p (g c) -> p g c", g=2)[:, :, o : o + 128]
                        nc.vector.tensor_mul(a3, a3, tri2r[:])
                    start = kb == 0
                    stop = kb == nkt - 1
                    blkA = slice((2 * p) * 128, (2 * p) * 128 + 128)
                    blkB = slice((2 * p + 1) * 128, (2 * p + 1) * 128 + 128)
                    nc.tensor.matmul(
                        outA[:, c0:],
                        lhsT=v_ext[kb][:, blkA],
                        rhs=att2[:, c0:QCHUNK],
                        start=start,
                        stop=stop,
                    )
                    nc.tensor.matmul(
                        outB[:, c0:],
                        lhsT=v_ext[kb][:, blkB],
                        rhs=att2[:, QCHUNK + c0 :],
                        start=start,
                        stop=stop,
                    )
                    if fillers and kb >= 2:
                        fillers.pop(0)()
                # gather sums into one tile (aligned sub-partition copies),
                # then one full-partition reciprocal: rows 0:64 = 1/sums_B,
                # rows 64:128 = 1/sums_A  (sub-partition recip_approx is broken)
                sc = misc_sb.tile([128, QCHUNK], f32, tag="sc", name="sums_sb")
                nc.vector.tensor_copy(sc[0:64, :], outB[0:64, :])
                nc.vector.tensor_copy(sc[64:128, :], outA[64:128, :])
                rec_raw = misc_sb.tile([128, QCHUNK], f32, tag="rec_raw", name="rec_raw")
                nc.vector.reciprocal_approx_fast(rec_raw[:], sc[:])
                # swap halves so divisors align with their heads' rows
                rec = misc_sb.tile([128, QCHUNK], f32, tag="rec", name="rec")
                nc.gpsimd.dma_start(rec[0:64, :], rec_raw[64:128, :])
                nc.gpsimd.dma_start(rec[64:128, :], rec_raw[0:64, :])
                cs = slice(j * QCHUNK, (j + 1) * QCHUNK)
                nc.vector.tensor_mul(
                    att_out[p][0:64, cs], outA[0:64, :], rec[0:64, :]
                )
                nc.vector.tensor_mul(
                    att_out[p][64:128, cs], outB[64:128, :], rec[64:128, :]
                )
                while fillers:
                    fillers.pop(0)()

            def proj_qt(qt, tail=False):
                # two half-units per q-tile so filler work lands evenly
                # between attention iterations (one 512-col output half each).
                # tail=True: PSUM->SBUF copies on the (idle-by-then) scalar
                # engine instead of the backlogged DVE, DMA out per half.
                state = {}

                def half(jc):
                    def emit():
                        if jc == 0:
                            state["ob"] = out_sb.tile([128, C], f32, tag="ob", name="ob")
                            state["ps2"] = ps.tile(
                                [128, 2 * QCHUNK], f32, tag="os", name="ps_proj"
                            )
                        ob, ps2 = state["ob"], state["ps2"]
                        for p in range(PAIRS):
                            nc.tensor.matmul(
                                ps2[:, jc * QCHUNK : (jc + 1) * QCHUNK],
                                lhsT=att_out[p][:, qt * 128 : (qt + 1) * 128],
                                rhs=wo[p][:, jc * QCHUNK : (jc + 1) * QCHUNK],
                                start=(p == 0),
                                stop=(p == PAIRS - 1),
                            )
                        obh = ob[:, jc * QCHUNK : (jc + 1) * QCHUNK]
                        psh = ps2[:, jc * QCHUNK : (jc + 1) * QCHUNK]
                        if tail:
                            nc.scalar.copy(obh, psh)
                            nc.sync.dma_start(
                                out_d[
                                    qt * 128 : (qt + 1) * 128,
                                    jc * QCHUNK : (jc + 1) * QCHUNK,
                                ],
                                obh,
                            )
                        else:
                            nc.vector.tensor_copy(obh, psh)
                            if jc == 1:
                                nc.sync.dma_start(
                                    out_d[qt * 128 : (qt + 1) * 128, :], ob[:]
                                )
                    return emit

                return [half(0), half(1)]

            # ---- emission schedule ----
            # prefix: first half of pair-0 Q/K + rope + v0-3 so the ACT exp
            # stream starts at ~t=22us; remaining QKV/V units are blocks at
            # chunk boundaries where they fill the PE idle of the ACT-paced
            # attention stream; proj rides as fillers in the last chunks.
            qkv_dsts = {
                ("q", 0): (wq, qt_raw[0], qt_r[0]),
                ("k", 0): (wk, kt_raw[0], kt_r[0]),
                ("q", 1): (wq, qt_raw[1], qt_r[1]),
                ("k", 1): (wk, kt_raw[1], kt_r[1]),
            }

            def U(which, p, tch):
                w, raw, _ = qkv_dsts[(which, p)]
                qkv_unit(w, raw, p, tch)

            def R(which, p, h):
                _, raw, roped = qkv_dsts[(which, p)]
                rope_half(raw, roped, h)

            U("q", 0, 0); U("q", 0, 1); U("k", 0, 0); U("k", 0, 1)
            R("q", 0, 0); R("k", 0, 0)
            for tt in range(0, 4):
                v_unit(tt)
            attn_chunk(0, 0)
            U("q", 0, 2); U("q", 0, 3); U("k", 0, 2); U("k", 0, 3)
            R("q", 0, 1); R("k", 0, 1)
            for tt in range(4, 8):
                v_unit(tt)
            attn_chunk(0, 1)
            U("q", 1, 0); U("q", 1, 1); U("k", 1, 0); U("k", 1, 1)
            R("q", 1, 0); R("k", 1, 0)
            for tt in range(8, 10):
                v_unit(tt)
            attn_chunk(1, 0)
            U("q", 1, 2); U("q", 1, 3); U("k", 1, 2); U("k", 1, 3)
            R("q", 1, 1); R("k", 1, 1)
            for tt in range(10, 12):
                v_unit(tt)
            attn_chunk(0, 2)
            for tt in range(12, 16):
                v_unit(tt)
            attn_chunk(1, 1)
            attn_chunk(0, 3, [f for qt in range(0, 4) for f in proj_qt(qt)])
            attn_chunk(1, 2, [f for qt in range(4, 8) for f in proj_qt(qt)])
            attn_chunk(1, 3, [f for qt in range(8, 12) for f in proj_qt(qt)])
            for qt in range(12, 16):
                for f in proj_qt(qt, tail=True):
                    f()

    nc.compile()
    return nc


def _prep_inputs(x, Wq, Wk, Wv, Wo, cos, sin):
    """Host-side sharding + layout prep. Returns list of per-core in_maps."""
    x = np.asarray(x, np.float32)
    Wq, Wk, Wv, Wo = (np.asarray(w, np.float32) for w in (Wq, Wk, Wv, Wo))
    cos, sin = np.asarray(cos, np.float32), np.asarray(sin, np.float32)

    # permute W rows to [evens; odds] within each head (rope pairing -> +-32)
    perm = np.concatenate(
        [
            np.concatenate(
                [np.arange(h * HD, (h + 1) * HD, 2), np.arange(h * HD + 1, (h + 1) * HD, 2)]
            )
            for h in range(H)
        ]
    )
    Wqp = Wq[perm]
    Wkp = Wk[perm]

    # rope maps [128, T] (identical for both heads of a pair, all cores)
    cosT = cos.T  # [32, T]
    sinT = sin.T
    cmap = np.empty((128, T), np.float32)
    smap = np.empty((128, T), np.float32)
    for blk in range(4):
        cmap[blk * 32 : (blk + 1) * 32] = cosT
        smap[blk * 32 : (blk + 1) * 32] = sinT if blk % 2 else -sinT
    cmap = cmap.astype(_bf16)
    smap = smap.astype(_bf16)

    xTb = [np.ascontiguousarray(x[b].T).astype(_bf16) for b in range(B)]

    in_maps = []
    for core in range(N_CORES):
        b, g = divmod(core, GROUPS)
        ms = slice(g * M_CORE, (g + 1) * M_CORE)
        in_maps.append(
            {
                "xt": xTb[b],
                "wqt": np.ascontiguousarray(Wqp[ms].T).astype(_bf16),
                "wkt": np.ascontiguousarray(Wkp[ms].T).astype(_bf16),
                "wvt": np.ascontiguousarray(Wv[ms].T).astype(_bf16),
                "wot": np.ascontiguousarray(Wo[:, ms].T).astype(_bf16),
                "cmap": cmap,
                "smap": smap,
            }
        )
    return in_maps


def _ensure_ntff_hook():
    """Install an antenv.axon_hooks shim so trace=True works in this
    container (the image's antenv lacks the axon_hooks module)."""
    import sys
    import types

    try:
        from antenv.axon_hooks import get_axon_ntff_profile_hook  # noqa: F401

        return
    except ImportError:
        pass
    sys.path.insert(0, "/root/.axon_site")
    from trn_agent_boot.trn_boot import _ntff_profile_via_ctypes

    hook = _ntff_profile_via_ctypes("/opt/axon/libaxon_pjrt.so")
    mod = types.ModuleType("antenv.axon_hooks")
    mod._hook = hook
    mod.get_axon_ntff_profile_hook = lambda: mod._hook
    mod.set_axon_ntff_profile_hook = lambda h: setattr(mod, "_hook", h)
    sys.modules["antenv.axon_hooks"] = mod

    # no bucket creds in this container; keep artifacts local
    import concourse.bass_utils as bu

    bu.upload_artifacts = lambda tmpdir: tmpdir


def kernel(x, Wq, Wk, Wv, Wo, cos, sin):
    global LAST_RESULTS
    from concourse.bass_utils import run_bass_kernel_spmd

    if "nc" not in _CACHE:
        _CACHE["nc"] = _build_bass()
    nc = _CACHE["nc"]

    in_maps = _prep_inputs(x, Wq, Wk, Wv, Wo, cos, sin)
    trace = bool(int(os.environ.get("KERNEL_TRACE", "0")))
    if trace:
        _ensure_ntff_hook()
    res = run_bass_kernel_spmd(
        nc, in_maps, core_ids=list(range(N_CORES)), trace=trace
    )
    LAST_RESULTS = res

    out = np.zeros((B, T, C), np.float32)
    for core in range(N_CORES):
        b = core // GROUPS
        out[b] += res.results[core]["out"]
    return out


# revision 11
# speedup vs baseline: 1.1187x; 1.1187x over previous
"""Trainium2 Bass kernel: causal multi-head attention with RoPE.

Problem: B=2, T=2048, C=1024, H=16, HD=64.
  q/k/v = x @ W{q,k,v}.T ; rope(q), rope(k)
  att = softmax(causal(q k^T / 8)) ; out = (att v) @ Wo.T

Sharding (8 cores): core i handles batch b = i//4 and head group g = i%4
(4 heads = 2 head-pairs, channel slice c in [256g, 256g+256)).
Each core computes its partial output x[b]-slice @ Wo[:, slice].T; the host
sums the 4 partials per batch (Wo row-parallel reduction done on host).

Device-side layout (per core), same math as the 193us baseline but with the
program restructured around the measured engine loads (PE 140us busy,
ACT 108us busy of which 28us was PSUM->SBUF copies):
  - ALL PSUM->SBUF copies moved from the scalar (ACT) engine to the vector
    engine; ACT now runs exp only (~80us).
  - QKV projection + RoPE emitted as per-512-column chunk units; attention
    chunks for BOTH head pairs are interleaved with those units so the ACT
    exp stream starts at ~t=17us and stays fed while the PE retires the
    remaining QKV/V work in the gaps of the ACT-paced attention stream.
  - PSUM plan: tag "big" [128,1024]f32 x2 bufs (scores st2 + QKV/V chunk
    accumulators) + tag "os" [128,1024]f32 x2 bufs (attV accumulators +
    projection accumulators) = exactly 8 banks.
  - xt is DMA'd in (column-chunk, ci) order so the first QKV chunk's
    operands arrive first.
  - v_ext ones-columns are a single strided memset; the V PSUM->SBUF
    scatter is 2 strided vector copies (even/odd heads).
Everything else (dual-half score matmuls, [V|1] ones-folding for softmax
denominators, triangular diagonal masks, no-max exp with scale=0.125,
host-side Wo row-parallel reduction) is unchanged from the baseline.
"""

import os

import numpy as np
import ml_dtypes

B, T, C, H, HD = 2, 2048, 1024, 16, 64
N_CORES = 8
GROUPS = 4  # head groups (of 4 heads) per batch
HPG = H // GROUPS  # heads per core = 4
M_CORE = HPG * HD  # 256 head channels per core
PAIRS = HPG // 2  # head pairs per core = 2
QCHUNK = 512  # q columns per attention chunk
KTILE = 128  # k rows per tile
NQC = T // QCHUNK  # 4
NT128 = T // 128  # 16

_bf16 = ml_dtypes.bfloat16

_CACHE = {}
LAST_RESULTS = None  # BassKernelResults of the most recent run (for test.py)


def _build_bass():
    """Trace the per-core Bass/Tile program (SPMD, same NEFF on all cores)."""
    from contextlib import ExitStack

    import concourse.bass as bass
    import concourse.tile as tile
    from concourse import bacc, mybir

    f32 = mybir.dt.float32
    bf16 = mybir.dt.bfloat16
    Exp = mybir.ActivationFunctionType.Exp

    nc = bacc.Bacc(
        "TRN2",
        target_bir_lowering=False,
        debug=False,
        enable_asserts=False,
        num_devices=N_CORES,
    )

    xt_d = nc.dram_tensor("xt", [C, T], bf16, kind="ExternalInput").ap()
    wq_d = nc.dram_tensor("wqt", [C, M_CORE], bf16, kind="ExternalInput").ap()
    wk_d = nc.dram_tensor("wkt", [C, M_CORE], bf16, kind="ExternalInput").ap()
    wv_d = nc.dram_tensor("wvt", [C, M_CORE], bf16, kind="ExternalInput").ap()
    wo_d = nc.dram_tensor("wot", [M_CORE, C], bf16, kind="ExternalInput").ap()
    cmap_d = nc.dram_tensor("cmap", [128, T], bf16, kind="ExternalInput").ap()
    smap_d = nc.dram_tensor("smap", [128, T], bf16, kind="ExternalInput").ap()
    out_d = nc.dram_tensor("out", [T, C], f32, kind="ExternalOutput").ap()

    NCT = C // 128  # 8 c-tiles

    with tile.TileContext(nc) as tc:
        with ExitStack() as ctx:
            consts = ctx.enter_context(tc.tile_pool(name="consts", bufs=1))
            qk_sb = ctx.enter_context(tc.tile_pool(name="qk_sb", bufs=1))
            rope_tmp = ctx.enter_context(tc.tile_pool(name="rope_tmp", bufs=2))
            att_sb = ctx.enter_context(tc.tile_pool(name="att_sb", bufs=4))
            misc_sb = ctx.enter_context(tc.tile_pool(name="misc_sb", bufs=2))
            out_sb = ctx.enter_context(tc.tile_pool(name="out_sb", bufs=4))
            ps = ctx.enter_context(tc.tile_pool(name="ps", bufs=2, space="PSUM"))

            # ---- DMA loads, in first-use order ----
            # wq first (first QKV units), then xt column-half 0 (2KB/partition
            # lines, full DMA rate), rope maps, wk, wv (v-tiles start early),
            # then xt half 1 and wo.
            wq = []
            for i in range(NCT):
                t = consts.tile([128, M_CORE], bf16, tag=f"wq{i}", name=f"wq{i}")
                nc.sync.dma_start(t[:], wq_d[i * 128 : (i + 1) * 128, :])
                wq.append(t)
            xt = [
                consts.tile([128, T], bf16, tag=f"xt{i}", name=f"xt{i}")
                for i in range(NCT)
            ]
            half = T // 2
            for i in range(NCT):
                nc.sync.dma_start(
                    xt[i][:, 0:half], xt_d[i * 128 : (i + 1) * 128, 0:half]
                )
            cmap = consts.tile([128, T], bf16, tag="cmap", name="cmap")
            nc.sync.dma_start(cmap[:], cmap_d[:])
            smap = consts.tile([128, T], bf16, tag="smap", name="smap")
            nc.sync.dma_start(smap[:], smap_d[:])
            wk = []
            for i in range(NCT):
                t = consts.tile([128, M_CORE], bf16, tag=f"wk{i}", name=f"wk{i}")
                nc.sync.dma_start(t[:], wk_d[i * 128 : (i + 1) * 128, :])
                wk.append(t)
            wv = []
            for i in range(NCT):
                t = consts.tile([128, M_CORE], bf16, tag=f"wv{i}", name=f"wv{i}")
                nc.sync.dma_start(t[:], wv_d[i * 128 : (i + 1) * 128, :])
                wv.append(t)
            for i in range(NCT):
                nc.sync.dma_start(
                    xt[i][:, half:T], xt_d[i * 128 : (i + 1) * 128, half:T]
                )
            wo = []
            for p in range(PAIRS):
                t = consts.tile([128, C], bf16, tag=f"wo{p}", name=f"wo{p}")
                nc.sync.dma_start(t[:], wo_d[p * 128 : (p + 1) * 128, :])
                wo.append(t)

            # upper-triangular (incl. diagonal) keep-mask, duplicated twice so
            # both heads' diagonal blocks mask with ONE strided DVE multiply:
            # tri2[p, g*128+y] = (p <= y)
            tri2 = consts.tile([128, 256], bf16, tag="tri2", name="tri2")
            nc.gpsimd.memset(tri2[:], 1.0)
            for g in range(2):
                nc.gpsimd.affine_select(
                    out=tri2[:, g * 128 : (g + 1) * 128],
                    in_=tri2[:, g * 128 : (g + 1) * 128],
                    compare_op=mybir.AluOpType.is_ge,
                    fill=0.0,
                    base=0,
                    pattern=[[1, 128]],
                    channel_multiplier=-1,
                )
            tri2r = tri2.rearrange("p (g y) -> p g y", g=2)

            # v_ext tiles: [128 k, 4*128]; head h occupies cols [h*128,(h+1)*128)
            # as [V_h | 1] for even h, [1 | V_h] for odd h.  Ones regions are
            # cols [64,192) u [320,448): one strided memset per tile.
            v_ext = []
            for tt in range(NT128):
                vt = qk_sb.tile([128, 4 * 128], bf16, tag=f"v{tt}", name=f"v{tt}")
                vr = vt.rearrange("p (a b) -> p a b", a=2)  # blocks at 0, 256
                nc.gpsimd.memset(vr[:, :, 64:192], 1.0)
                v_ext.append(vt)

            # ---- per-chunk QKV projection + RoPE unit emitters ----
            # raw/roped QT,KT tiles [128 m, T]; m rows: head-local channel
            # hr = r % 64, j = hr % 32, parity = hr // 32 (host permuted W rows
            # to [evens; odds] per head so rope pairing is a +-32 row shift).
            qt_raw, kt_raw, qt_r, kt_r = {}, {}, {}, {}
            for p in range(PAIRS):
                qt_raw[p] = qk_sb.tile([128, T], bf16, tag=f"qtraw{p}", name=f"qtraw{p}")
                kt_raw[p] = qk_sb.tile([128, T], bf16, tag=f"ktraw{p}", name=f"ktraw{p}")
                qt_r[p] = qk_sb.tile([128, T], bf16, tag=f"qtr{p}", name=f"qtr{p}")
                kt_r[p] = qk_sb.tile([128, T], bf16, tag=f"ktr{p}", name=f"ktr{p}")

            def qkv_unit(w, raw, p, tch):
                """One 512-col chunk: 8 accumulating matmuls -> DVE copy."""
                cs = slice(tch * QCHUNK, (tch + 1) * QCHUNK)
                psq = ps.tile([128, QCHUNK], f32, tag="big", name="ps_qk")
                for ci in range(NCT):
                    nc.tensor.matmul(
                        psq[:],
                        lhsT=w[ci][:, p * 128 : (p + 1) * 128],
                        rhs=xt[ci][:, cs],
                        start=(ci == 0),
                        stop=(ci == NCT - 1),
                    )
                nc.vector.tensor_copy(raw[:, cs], psq[:])

            def rope_full(raw, roped):
                """Whole-tensor RoPE: 32-row block swap via gpsimd dma (its
                own DMASW queues, off the input-load DMAHW path) + 2 muls +
                add on the DVE."""
                shf = rope_tmp.tile([128, T], bf16, tag="shf", name="shf")
                for dst_b, src_b in ((0, 1), (1, 0), (2, 3), (3, 2)):
                    nc.gpsimd.dma_start(
                        shf[dst_b * 32 : (dst_b + 1) * 32, :],
                        raw[src_b * 32 : (src_b + 1) * 32, :],
                    )
                t1 = rope_tmp.tile([128, T], bf16, tag="t1", name="rope_t1")
                nc.vector.tensor_mul(t1[:], raw[:], cmap[:])
                t2 = rope_tmp.tile([128, T], bf16, tag="t2", name="rope_t2")
                nc.vector.tensor_mul(t2[:], shf[:], smap[:])
                nc.vector.tensor_add(roped[:], t1[:], t2[:])

            def v_unit(tt):
                """One V k-tile: 8 accumulating matmuls -> 2 strided DVE
                copies into the [V|1] layout."""
                vt = v_ext[tt]
                psv = ps.tile([128, M_CORE], f32, tag="big", name="ps_v")
                for ci in range(NCT):
                    nc.tensor.matmul(
                        psv[:],
                        lhsT=xt[ci][:, tt * 128 : (tt + 1) * 128],
                        rhs=wv[ci][:],
                        start=(ci == 0),
                        stop=(ci == NCT - 1),
                    )
                pr = psv.rearrange("p (a b) -> p a b", a=2)  # head blocks 0,128
                vr = vt.rearrange("p (a b) -> p a b", a=2)  # dst blocks 0,256
                nc.vector.tensor_copy(vr[:, :, 0:64], pr[:, :, 0:64])  # heads 0,2
                nc.vector.tensor_copy(vr[:, :, 192:256], pr[:, :, 64:128])  # 1,3

            # ---- attention (per head pair, per q chunk) ----
            att_out = []
            for p in range(PAIRS):
                ao = qk_sb.tile([128, T], bf16, tag=f"ao{p}", name=f"ao{p}")
                att_out.append(ao)

            def attn_chunk(p, j, fillers=None):
                os2 = ps.tile([128, 2 * QCHUNK], f32, tag="os", name="ps_os")
                outA = os2[:, 0:QCHUNK]   # rows 0:64 attV_A, 64:128 sums_A
                outB = os2[:, QCHUNK:]    # rows 0:64 sums_B, 64:128 attV_B
                nkt = (j + 1) * (QCHUNK // KTILE)
                for kb in range(nkt):
                    o = KTILE * kb - QCHUNK * j
                    c0 = max(o, 0)
                    qs = slice(j * QCHUNK + c0, (j + 1) * QCHUNK)
                    ks = slice(kb * KTILE, (kb + 1) * KTILE)
                    # both heads' scores in one 2-bank tile -> single exp
                    st2 = ps.tile([128, 2 * QCHUNK], f32, tag="big", name="ps_st")
                    nc.tensor.matmul(
                        st2[:, c0:QCHUNK],
                        lhsT=kt_r[p][0:64, ks],
                        rhs=qt_r[p][0:64, qs],
                        start=True,
                        stop=True,
                        tile_position=(0, 0),
                    )
                    nc.tensor.matmul(
                        st2[:, QCHUNK + c0 :],
                        lhsT=kt_r[p][64:128, ks],
                        rhs=qt_r[p][64:128, qs],
                        start=True,
                        stop=True,
                        tile_position=(64, 0),
                    )
                    att2 = att_sb.tile([128, 2 * QCHUNK], bf16, tag="att", name="att2")
                    # single exp across both banks; the [QCHUNK, QCHUNK+c0)
                    # gap holds stale-but-finite scores and is never read
                    nc.scalar.activation(att2[:, c0:], st2[:, c0:], Exp, scale=0.125)
                    if o >= 0:  # diagonal tile: triangular mask, both heads at once
                        a3 = att2.rearrange("p (g c) -> p g c", g=2)[:, :, o : o + 128]
                        nc.vector.tensor_mul(a3, a3, tri2r[:])
                    start = kb == 0
                    stop = kb == nkt - 1
                    blkA = slice((2 * p) * 128, (2 * p) * 128 + 128)
                    blkB = slice((2 * p + 1) * 128, (2 * p + 1) * 128 + 128)
                    nc.tensor.matmul(
                        outA[:, c0:],
                        lhsT=v_ext[kb][:, blkA],
                        rhs=att2[:, c0:QCHUNK],
                        start=start,
                        stop=stop,
                    )
                    nc.tensor.matmul(
                        outB[:, c0:],
                        lhsT=v_ext[kb][:, blkB],
                        rhs=att2[:, QCHUNK + c0 :],
                        start=start,
                        stop=stop,
                    )
                    if fillers and kb >= 2:
                        fillers.pop(0)()
                # gather sums into one tile (aligned sub-partition copies),
                # then one full-partition reciprocal: rows 0:64 = 1/sums_B,
                # rows 64:128 = 1/sums_A  (sub-partition recip_approx is broken)
                sc = misc_sb.tile([128, QCHUNK], f32, tag="sc", name="sums_sb")
                nc.vector.tensor_copy(sc[0:64, :], outB[0:64, :])
                nc.vector.tensor_copy(sc[64:128, :], outA[64:128, :])
                rec_raw = misc_sb.tile([128, QCHUNK], f32, tag="rec_raw", name="rec_raw")
                nc.vector.reciprocal_approx_fast(rec_raw[:], sc[:])
                # swap halves so divisors align with their heads' rows
                rec = misc_sb.tile([128, QCHUNK], f32, tag="rec", name="rec")
                nc.gpsimd.dma_start(rec[0:64, :], rec_raw[64:128, :])
                nc.gpsimd.dma_start(rec[64:128, :], rec_raw[0:64, :])
                cs = slice(j * QCHUNK, (j + 1) * QCHUNK)
                nc.vector.tensor_mul(
                    att_out[p][0:64, cs], outA[0:64, :], rec[0:64, :]
                )
                nc.vector.tensor_mul(
                    att_out[p][64:128, cs], outB[64:128, :], rec[64:128, :]
                )
                while fillers:
                    fillers.pop(0)()

            def proj_qt(qt, tail=False):
                # two half-units per q-tile so filler work lands evenly
                # between attention iterations (one 512-col output half each).
                # tail=True: PSUM->SBUF copies on the (idle-by-then) scalar
                # engine instead of the backlogged DVE, DMA out per half.
                state = {}

                def half(jc):
                    def emit():
                        if jc == 0:
                            state["ob"] = out_sb.tile([128, C], f32, tag="ob", name="ob")
                            state["ps2"] = ps.tile(
                                [128, 2 * QCHUNK], f32, tag="os", name="ps_proj"
                            )
                        ob, ps2 = state["ob"], state["ps2"]
                        for p in range(PAIRS):
                            nc.tensor.matmul(
                                ps2[:, jc * QCHUNK : (jc + 1) * QCHUNK],
                                lhsT=att_out[p][:, qt * 128 : (qt + 1) * 128],
                                rhs=wo[p][:, jc * QCHUNK : (jc + 1) * QCHUNK],
                                start=(p == 0),
                                stop=(p == PAIRS - 1),
                            )
                        obh = ob[:, jc * QCHUNK : (jc + 1) * QCHUNK]
                        psh = ps2[:, jc * QCHUNK : (jc + 1) * QCHUNK]
                        if tail:
                            nc.scalar.copy(obh, psh)
                            nc.sync.dma_start(
                                out_d[
                                    qt * 128 : (qt + 1) * 128,
                                    jc * QCHUNK : (jc + 1) * QCHUNK,
                                ],
                                obh,
                            )
                        else:
                            nc.vector.tensor_copy(obh, psh)
                            if jc == 1:
                                nc.sync.dma_start(
                                    out_d[qt * 128 : (qt + 1) * 128, :], ob[:]
                                )
                    return emit

                return [half(0), half(1)]

            # ---- emission schedule ----
            # prefix: first half of pair-0 Q/K + rope + v0-3 so the ACT exp
            # stream starts at ~t=22us; remaining QKV/V units are blocks at
            # chunk boundaries where they fill the PE idle of the ACT-paced
            # attention stream; proj rides as fillers in the last chunks.
            qkv_dsts = {
                ("q", 0): (wq, qt_raw[0], qt_r[0]),
                ("k", 0): (wk, kt_raw[0], kt_r[0]),
                ("q", 1): (wq, qt_raw[1], qt_r[1]),
                ("k", 1): (wk, kt_raw[1], kt_r[1]),
            }

            def U(which, p, tch):
                w, raw, _ = qkv_dsts[(which, p)]
                qkv_unit(w, raw, p, tch)

            def R(which, p):
                _, raw, roped = qkv_dsts[(which, p)]
                rope_full(raw, roped)

            # pair-0 Q/K + rope (latency covered by v0-7 PE work), then the
            # pair-0 chunks big-first so their long exp streams cover pair-1
            # QKV + rope on the PE; pair-1 chunks descending with proj fillers
            # chaining (1,j)+(0,j) -> proj 4j..4j+3; tail is the smallest
            # chunk's q-range.
            for tch in range(NQC):
                U("q", 0, tch)
            for tch in range(NQC):
                U("k", 0, tch)
            R("q", 0); R("k", 0)
            for tt in range(0, 8):
                v_unit(tt)
            attn_chunk(0, 1)
            for tt in range(8, 16):
                v_unit(tt)
            attn_chunk(0, 3)
            for tch in range(NQC):
                U("q", 1, tch)
            attn_chunk(0, 2)
            for tch in range(NQC):
                U("k", 1, tch)
            R("q", 1); R("k", 1)
            attn_chunk(0, 0)
            attn_chunk(1, 0)
            attn_chunk(1, 1, [f for qt in range(0, 4) for f in proj_qt(qt)])
            attn_chunk(1, 2, [f for qt in range(4, 8) for f in proj_qt(qt)])
            attn_chunk(1, 3, [f for qt in range(8, 12) for f in proj_qt(qt)])
            for qt in range(12, 16):
                for f in proj_qt(qt, tail=True):
                    f()

    nc.compile()
    return nc


def _prep_inputs(x, Wq, Wk, Wv, Wo, cos, sin):
    """Host-side sharding + layout prep. Returns list of per-core in_maps."""
    x = np.asarray(x, np.float32)
    Wq, Wk, Wv, Wo = (np.asarray(w, np.float32) for w in (Wq, Wk, Wv, Wo))
    cos, sin = np.asarray(cos, np.float32), np.asarray(sin, np.float32)

    # permute W rows to [evens; odds] within each head (rope pairing -> +-32)
    perm = np.concatenate(
        [
            np.concatenate(
                [np.arange(h * HD, (h + 1) * HD, 2), np.arange(h * HD + 1, (h + 1) * HD, 2)]
            )
            for h in range(H)
        ]
    )
    Wqp = Wq[perm]
    Wkp = Wk[perm]

    # rope maps [128, T] (identical for both heads of a pair, all cores)
    cosT = cos.T  # [32, T]
    sinT = sin.T
    cmap = np.empty((128, T), np.float32)
    smap = np.empty((128, T), np.float32)
    for blk in range(4):
        cmap[blk * 32 : (blk + 1) * 32] = cosT
        smap[blk * 32 : (blk + 1) * 32] = sinT if blk % 2 else -sinT
    cmap = cmap.astype(_bf16)
    smap = smap.astype(_bf16)

    xTb = [np.ascontiguousarray(x[b].T).astype(_bf16) for b in range(B)]

    in_maps = []
    for core in range(N_CORES):
        b, g = divmod(core, GROUPS)
        ms = slice(g * M_CORE, (g + 1) * M_CORE)
        in_maps.append(
            {
                "xt": xTb[b],
                "wqt": np.ascontiguousarray(Wqp[ms].T).astype(_bf16),
                "wkt": np.ascontiguousarray(Wkp[ms].T).astype(_bf16),
                "wvt": np.ascontiguousarray(Wv[ms].T).astype(_bf16),
                "wot": np.ascontiguousarray(Wo[:, ms].T).astype(_bf16),
                "cmap": cmap,
                "smap": smap,
            }
        )
    return in_maps


def _ensure_ntff_hook():
    """Install an antenv.axon_hooks shim so trace=True works in this
    container (the image's antenv lacks the axon_hooks module)."""
    import sys
    import types

    try:
        from antenv.axon_hooks import get_axon_ntff_profile_hook  # noqa: F401

        return
    except ImportError:
        pass
    sys.path.insert(0, "/root/.axon_site")
    from trn_agent_boot.trn_boot import _ntff_profile_via_ctypes

    hook = _ntff_profile_via_ctypes("/opt/axon/libaxon_pjrt.so")
    mod = types.ModuleType("antenv.axon_hooks")
    mod._hook = hook
    mod.get_axon_ntff_profile_hook = lambda: mod._hook
    mod.set_axon_ntff_profile_hook = lambda h: setattr(mod, "_hook", h)
    sys.modules["antenv.axon_hooks"] = mod

    # no bucket creds in this container; keep artifacts local
    import concourse.bass_utils as bu

    bu.upload_artifacts = lambda tmpdir: tmpdir


def kernel(x, Wq, Wk, Wv, Wo, cos, sin):
    global LAST_RESULTS
    from concourse.bass_utils import run_bass_kernel_spmd

    if "nc" not in _CACHE:
        _CACHE["nc"] = _build_bass()
    nc = _CACHE["nc"]

    in_maps = _prep_inputs(x, Wq, Wk, Wv, Wo, cos, sin)
    trace = bool(int(os.environ.get("KERNEL_TRACE", "0")))
    if trace:
        _ensure_ntff_hook()
    res = run_bass_kernel_spmd(
        nc, in_maps, core_ids=list(range(N_CORES)), trace=trace
    )
    LAST_RESULTS = res

    out = np.zeros((B, T, C), np.float32)
    for core in range(N_CORES):
        b = core // GROUPS
        out[b] += res.results[core]["out"]
    return out
